# revision 18
# baseline (speedup 1.0000x reference)
"""Trainium2 Bass kernel for the tanh-RNN problem (v2: r-matmul restructure).

Reference computation (per batch row b):
    xproj = input @ wi + brec                 # [B, T, H]
    h_{t+1} = 0.5*h_t + 0.5*tanh(h_t @ wrec.T + xproj_t)
    output  = hs @ wo                         # [B, T, O]

v2 structure (8 cores, data-parallel over batch, B_local = 8):
  State g = 2*h kept H-major ([H, BL] fp16). Using
      z_t = W' g_t + x_t,  r_t = tanh(z_t),  g_{t+1} = 0.5 g_t + r_t
  expand  z_t = (W'/2) g_{t-1} + W' r_{t-1} + x_t  so each step's matmuls
  consume g_{t-1} (old) and r_{t-1} (tanh output, fp16 straight from ACT).
  The DVE blend g_t = 0.5 g_{t-1} + r_{t-1} is OFF the critical chain
  (needed only two steps later). PSUM banks alternate by step parity so the
  ACT read of step t never serializes against the matmul writes of step t+1.

  Per step: 32 matmuls (4 m-tiles x 4 k-tiles x {g-pass, r-pass}), 2 ACT
  tanh (m-pairs, 3D AP over two PSUM banks -> fp16 r), 2 DVE fused blends.
  xproj is preloaded into PSUM by strided chunk matmuls (32 steps/chunk,
  16 steps per parity bank); brec enters via an augmented ones-row of x.
  Output projection at the tail from the fp16 history buffer HT.
"""

import numpy as np

import concourse.bacc as bacc
import concourse.mybir as mybir
from concourse.tile import TileContext, add_dep_helper
from concourse import bass_utils

F16 = mybir.dt.float16
F32 = mybir.dt.float32

B, T_FULL, I, H, O = 64, 1024, 64, 512, 64
NCORES = 8
BL = B // NCORES          # 8 batch rows per core
KT = H // 128             # 4 tiles over H
CHUNK = 64                # steps per psum chunk (32 per parity per bank)
PCOLS = (CHUNK // 2) * BL  # 256 cols per m-half of a pair bank


def _mm_order():
    """(m, k, is_rpass) slots.

    All 16 g-pass matmuls first (they consume ancient state), then the
    r-pass ordered so pair01's matmuls finish by slot 23 (its ACT gates
    step t+1's r01 consumption at slot 16) and r23(t-1) is not consumed
    before slot 20 (its ACT lands late in step t-1).
    """
    order = [(m, k, 0) for m in range(KT) for k in range(KT)]
    order += [(0, 0, 1), (0, 1, 1), (1, 0, 1), (1, 1, 1)]
    order += [(2, 0, 1), (2, 1, 1), (3, 0, 1), (3, 1, 1)]
    order += [(0, 2, 1), (0, 3, 1), (1, 2, 1), (1, 3, 1)]
    order += [(2, 2, 1), (2, 3, 1), (3, 2, 1), (3, 3, 1)]
    return order


MM_ORDER = _mm_order()
PAIR01_LAST = 27   # slot of pair01's final matmul (m1 k3 r-pass)
PAIR23_LAST = 31


def build(t_steps: int = T_FULL):
    assert t_steps % CHUNK == 0
    nchunk = t_steps // CHUNK
    cols = t_steps * BL
    htw = (t_steps + 2) * BL       # per-m width of history buffer

    nc = bacc.Bacc("TRN2", target_bir_lowering=False, debug=False)
    pe_prev = [None]

    def mm(*args, **kw):
        inst = nc.tensor.matmul(*args, **kw)
        if pe_prev[0] is not None:
            add_dep_helper(inst.ins, pe_prev[0].ins, sync=False, reason="pe order")
        pe_prev[0] = inst
        return inst

    d_wT = nc.dram_tensor("wT", [KT, 128, H], F16, kind="ExternalInput")    # wrec.T/2
    d_wT4 = nc.dram_tensor("wT4", [KT, 128, H], F16, kind="ExternalInput")  # wrec.T/4
    d_wi = nc.dram_tensor("wiA", [I + 1, H], F16, kind="ExternalInput")
    d_wo = nc.dram_tensor("woT", [KT, 128, O], F16, kind="ExternalInput")   # wo/2
    d_g0 = nc.dram_tensor("g0", [KT, 128, 2 * BL], F16, kind="ExternalInput")
    d_xT = nc.dram_tensor("xT", [I + 1, cols], F16, kind="ExternalInput")
    d_out = nc.dram_tensor("outT", [O, cols], F32, kind="ExternalOutput")

    with TileContext(nc) as tc:
        with (
            tc.tile_pool(name="wpool", bufs=1) as wpool,
            tc.tile_pool(name="ht", bufs=1) as htpool,
            tc.tile_pool(name="r", bufs=1) as rpool,
            tc.tile_pool(name="osb", bufs=4) as opool,
            tc.tile_pool(name="px", bufs=1, space="PSUM") as px,
        ):
            wT = [wpool.tile([128, H], F16, tag=f"wT{k}", name=f"wT{k}") for k in range(KT)]
            wT4 = [wpool.tile([128, H], F16, tag=f"wT4{k}", name=f"wT4{k}") for k in range(KT)]
            for k in range(KT):
                nc.sync.dma_start(wT[k][:], d_wT[k])
                nc.sync.dma_start(wT4[k][:], d_wT4[k])
            wi = wpool.tile([I + 1, H], F16, tag="wi")
            nc.sync.dma_start(wi[:], d_wi[:])
            wo = [wpool.tile([128, O], F16, tag=f"wo{k}", name=f"wo{k}") for k in range(KT)]
            for k in range(KT):
                nc.sync.dma_start(wo[k][:], d_wo[k])
            xT = wpool.tile([I + 1, cols], F16, tag="xT")
            nc.sync.dma_start(xT[:], d_xT[:])

            # history: m-major blocks; col (s+1)*BL holds g_s (s = -1..t_steps)
            HT = htpool.tile([128, KT * htw], F16, tag="HT")
            for m in range(KT):
                nc.sync.dma_start(HT[:, m * htw : m * htw + 2 * BL], d_g0[m])

            # xT viewed as [65, chunk, q(=16), parity, b]: step t = c*CHUNK + 2q + p
            xTr = xT.rearrange(
                "p (c q par b) -> p c q par b", c=nchunk, q=CHUNK // 2, par=2, b=BL
            )

            # Persistent rings (no pool rotation -> no release instructions,
            # which would head-of-line block the ACT queue waiting on
            # next-step matmuls). psum: one bank per (chunk-parity,
            # step-parity, m-pair) = all 8 banks; the ACT read of one pair
            # bank never conflicts with matmul writes of the other pair.
            px_ring = [
                [[px.tile([128, 2 * PCOLS], F32, tag=f"px{cp}{p}{pr}", name=f"px{cp}{p}{pr}")
                  for pr in range(2)]
                 for p in range(2)]
                for cp in range(2)
            ]
            RDEPTH = 4
            r_ring = [
                [rpool.tile([128, 2 * BL], F16, tag=f"rr{pair}{sp}", name=f"rr{pair}{sp}")
                 for sp in range(RDEPTH)]
                for pair in range(2)
            ]

            def refill(c, p):
                """xproj chunk c, parity p -> the two pair psum banks."""
                rhs = xTr[:, c, :, p, :]      # [65, 32, 8] strided columns
                for pr in range(2):
                    pt = px_ring[c % 2][p][pr]
                    for mi in range(2):
                        # start=True clears has_written BANK-wide: only the
                        # first half may use it
                        mm(
                            pt[:, mi * PCOLS : (mi + 1) * PCOLS],
                            lhsT=wi[:, (2 * pr + mi) * 128 : (2 * pr + mi + 1) * 128],
                            rhs=rhs,
                            start=(mi == 0),
                            stop=False,
                            skip_group_check=True,
                        )

            refill(0, 0)
            refill(0, 1)
            r_prev = None                         # [r01, r23] of step t-1
            for t in range(t_steps):
                c = t // CHUNK
                tt = t % CHUNK
                p = tt % 2
                q = tt // 2                       # column group in parity bank
                if tt == 1 and c + 1 < nchunk:
                    refill(c + 1, 0)
                    refill(c + 1, 1)
                bands = px_ring[c % 2][p]     # [pair01 bank, pair23 bank]

                r01 = r_ring[0][t % 4]
                r23 = r_ring[1][t % 4]
                r_new = [r01, r23]

                def psum_slice(m):
                    return bands[m // 2][:, (m % 2) * PCOLS + q * BL : (m % 2) * PCOLS + (q + 1) * BL]

                for slot, (m, k, is_r) in enumerate(MM_ORDER):
                    if not (is_r and r_prev is None):
                        if is_r:
                            rhs = r_prev[k // 2][:, (k % 2) * BL : (k % 2 + 1) * BL]
                            lhsT = wT[k][:, m * 128 : (m + 1) * 128]
                        else:
                            rhs = HT[:, k * htw + t * BL : k * htw + (t + 1) * BL]
                            lhsT = wT4[k][:, m * 128 : (m + 1) * 128]
                        mm(
                            psum_slice(m),
                            lhsT=lhsT,
                            rhs=rhs,
                            start=False,
                            stop=False,
                            skip_group_check=True,
                        )
                    if slot == PAIR01_LAST:
                        _act_dve(nc, bands[0], HT, r01, 0, q, t, htw)
                    elif slot == PAIR23_LAST:
                        _act_dve(nc, bands[1], HT, r23, 1, q, t, htw)
                r_prev = r_new

            # ---- output projection tail: outT = (wo/2).T @ g ----
            OC = min(512, cols)                   # output chunk columns
            # alias two recurrence banks (idle by the tail) as accumulators
            po_ring = [px_ring[0][0][0][:O, :OC], px_ring[0][1][0][:O, :OC]]
            for c in range(cols // OC):
                po = po_ring[c % 2]
                for k in range(KT):
                    mm(
                        po[:],
                        lhsT=wo[k][:],
                        rhs=HT[:, k * htw + 2 * BL + c * OC : k * htw + 2 * BL + (c + 1) * OC],
                        start=(k == 0),
                        stop=(k == KT - 1),
                    )
                ot = opool.tile([O, OC], F32, tag="osb", name=f"ot{c}")
                nc.vector.tensor_copy(ot[:], po[:])
                nc.sync.dma_start(d_out[:, c * OC : (c + 1) * OC], ot[:])

    _thin_pe_clock(nc)
    nc.compile()
    return nc


def _thin_pe_clock(nc):
    """Strip unreferenced PE engine-clock increments from the BIR.

    Tile attaches a sem-inc to EVERY matmul; the semaphore-update pipeline
    sustains only ~34ns/inc, so at 32 matmuls/step the inc stream (not the
    PE) becomes the step clock. Since the PE executes in order, an inc at
    tick v implies all earlier matmuls completed -- keeping increments only
    at ticks some wait references (and remapping waits to their rank) is
    semantically equivalent and takes the inc stream off the critical path.
    """
    import bisect

    fn = nc.m.functions[0]
    SEM = None
    # identify the PE clock sem (the one PE instructions sem-inc)
    for blk in fn.blocks:
        for inst in blk.instructions:
            si = inst.sync_info
            if si is None:
                continue
            for u in si.on_update:
                if u.ant_name and u.ant_name.startswith("PE_") and u.update_mode == "sem-inc":
                    SEM = u.id
                    break
            if SEM is not None:
                break
        if SEM is not None:
            break
    if SEM is None:
        return
    refs = set()
    for blk in fn.blocks:
        for inst in blk.instructions:
            si = inst.sync_info
            if si is None:
                continue
            for w in si.on_wait:
                if w.id == SEM:
                    assert w.wait_mode == "sem-ge-imm", w.wait_mode
                    refs.add(w.wait_value)
    kept = sorted(refs)
    tick = 0
    for blk in fn.blocks:
        for inst in blk.instructions:
            si = inst.sync_info
            if si is None:
                continue
            ups = list(si.on_update)
            has = [u for u in ups if u.id == SEM]
            if has:
                assert len(has) == 1 and has[0].update_value == 1
                tick += 1
                if tick not in refs:
                    si.on_update = [u for u in ups if u.id != SEM]
    for blk in fn.blocks:
        for inst in blk.instructions:
            si = inst.sync_info
            if si is None:
                continue
            for w in si.on_wait:
                if w.id == SEM:
                    w.wait_value = bisect.bisect_right(kept, w.wait_value)


def _act_dve(nc, band, HT, r_tile, pair, q, t, htw):
    """tanh of psum pair bank -> fp16 r; fused blend g_{t+1} = 0.5 g_t + r_t."""
    m0 = 2 * pair
    pin = band.rearrange("p (m c) -> p m c", c=PCOLS)[:, :, q * BL : (q + 1) * BL]
    nc.scalar.activation(
        r_tile.rearrange("p (m b) -> p m b", b=BL)[:],
        pin,
        mybir.ActivationFunctionType.Tanh,
    )
    ht3 = HT.rearrange("p (m w) -> p m w", w=htw)
    nc.vector.scalar_tensor_tensor(
        ht3[:, m0 : m0 + 2, (t + 2) * BL : (t + 3) * BL],
        in0=ht3[:, m0 : m0 + 2, (t + 1) * BL : (t + 2) * BL],
        scalar=0.5,
        in1=r_tile.rearrange("p (m b) -> p m b", b=BL)[:],
        op0=mybir.AluOpType.mult,
        op1=mybir.AluOpType.add,
    )


_CACHE = {}


def _get_nc(t_steps):
    if t_steps not in _CACHE:
        _CACHE[t_steps] = build(t_steps)
    return _CACHE[t_steps]


def prep_inputs(input, wi, wrec, wo, brec, h0, t_steps):
    """Host-side shard + layout prep. Returns list of 8 in_maps."""
    input = np.asarray(input, dtype=np.float32)
    wi = np.asarray(wi, dtype=np.float32)
    wrec = np.asarray(wrec, dtype=np.float32)
    wo = np.asarray(wo, dtype=np.float32)
    brec = np.asarray(brec, dtype=np.float32)
    h0 = np.asarray(h0, dtype=np.float32)

    wT = np.ascontiguousarray((wrec.T / 2.0).astype(np.float16).reshape(KT, 128, H))
    wT4 = np.ascontiguousarray((wrec.T / 4.0).astype(np.float16).reshape(KT, 128, H))
    wiA = np.concatenate([wi, brec[None, :]], axis=0).astype(np.float16)
    woT = np.ascontiguousarray((wo / 2.0).astype(np.float16).reshape(KT, 128, O))
    g0 = np.empty((H, 2 * BL), np.float32)
    g0[:, :BL] = 4.0 * h0[:, None]     # g_{-1} = 2*g_0 (feeds the W'/2 pass)
    g0[:, BL:] = 2.0 * h0[:, None]     # g_0
    g0 = np.ascontiguousarray(g0.astype(np.float16).reshape(KT, 128, 2 * BL))

    in_maps = []
    for c in range(NCORES):
        xc = input[c * BL : (c + 1) * BL, :t_steps, :]
        xT = np.ascontiguousarray(np.transpose(xc, (2, 1, 0)).reshape(I, t_steps * BL))
        xA = np.concatenate(
            [xT, np.ones((1, t_steps * BL), np.float32)], axis=0
        ).astype(np.float16)
        in_maps.append({"wT": wT, "wT4": wT4, "wiA": wiA, "woT": woT, "g0": g0, "xT": xA})
    return in_maps


def run_sharded(inputs, t_steps=T_FULL, trace=False):
    nc = _get_nc(t_steps)
    in_maps = prep_inputs(**inputs, t_steps=t_steps)
    res = bass_utils.run_bass_kernel_spmd(
        nc, in_maps, core_ids=list(range(NCORES)), trace=trace
    )
    outs = []
    for c in range(NCORES):
        oT = res.results[c]["outT"]
        outs.append(np.transpose(oT.reshape(O, t_steps, BL), (2, 1, 0)))
    return np.concatenate(outs, axis=0).astype(np.float32), res


def kernel(input, wi, wrec, wo, brec, h0):
    out, _ = run_sharded(
        dict(input=input, wi=wi, wrec=wrec, wo=wo, brec=brec, h0=h0),
        t_steps=T_FULL,
        trace=False,
    )
    return out
